# revision 22
# baseline (speedup 1.0000x reference)
"""Trainium2 Bass kernel for nn_Critic (bidirectional-LSTM critic network).

Data-parallel over the B (sequence) dimension: 8 NeuronCores x 512 sequences.
Per core, feature-major layout throughout:

  Phase T (trunk): two LayerNorm-MLP layers. Mean-centering is folded into
    the weights on the host (W @ (I - 1/64)), so LN reduces to an rsqrt of
    the per-sample sum-of-squares, computed with PE reduce/replicate matmuls.
    Timestep blocks are emitted two-ended (t, 63-t) so both LSTM directions
    could stream (phases are serialized in v1 due to ACT table sets).
  Phase L (LSTM): 64 steps, fw+bw packed on partitions [128 = 64fw|64bw, 512].
    Gates come from col-tiled matmuls; sigmoid/tanh on ACT; cell update on
    DVE; per-step head dot-products (wx, wp) via tiny matmuls, staged to DRAM.
  Phase H (head): per-row LayerNorm over T, softmax(pn @ W3) attention,
    weighted sum -> [2B] output.
"""

import sys

for _p in ("/opt/trn_rl_repo",):
    if _p not in sys.path:
        sys.path.insert(0, _p)

import json as _json
from types import MethodType as _MethodType

import numpy as np

import concourse.bass as bass
import concourse.tile as tile
from concourse import mybir
from concourse.bass_utils import run_bass_kernel_spmd

F32 = mybir.dt.float32
AF = mybir.ActivationFunctionType
ALU = mybir.AluOpType

B, T, H, OBS, ACTD = 4096, 64, 64, 128, 32
NCORES = 8
BC = B // NCORES  # 512 sequences per core
NT = T * BC  # 32768 rows per core
EPS = 1e-12

# ---------------------------------------------------------------- waitfix --
# This walrus build rejects instructions carrying more than one sync-wait
# command. The Tile kernel-tail drain (and barriers) routinely carry more.
# Patch the serialized BIR: move excess waits onto inserted NoOp carriers.
_MAX_WAITS = 1


def _patch_bir(bir):
    n = [0]

    def fresh():
        n[0] += 1
        return f"I-waitfix-{n[0]}"

    for fn in bir.get("functions", []):
        for bb in fn.get("blocks", []):
            out = []
            for inst in bb.get("instructions", []):
                si = inst.get("sync_info") or {}
                waits = si.get("on_wait") or []
                if len(waits) > _MAX_WAITS:
                    extra = waits[: len(waits) - _MAX_WAITS]
                    keep = waits[len(waits) - _MAX_WAITS :]
                    for i in range(0, len(extra), _MAX_WAITS):
                        out.append(
                            {
                                "name": fresh(),
                                "opcode": "NoOp",
                                "engine": inst["engine"],
                                "ins": [],
                                "outs": [],
                                "sync_info": {
                                    "on_wait": extra[i : i + _MAX_WAITS],
                                    "on_update": [],
                                },
                            }
                        )
                    si = dict(si)
                    si["on_wait"] = keep
                    inst = dict(inst)
                    inst["sync_info"] = si
                out.append(inst)
            bb["instructions"] = out
    return bir


def _install_waitfix(nc):
    orig = nc.to_json_bytes

    def patched(self):
        return _json.dumps(_patch_bir(_json.loads(orig()))).encode()

    nc.to_json_bytes = _MethodType(patched, nc)
    return nc


# ----------------------------------------------------------- host weights --


def _prep_consts(p):
    """Fold reference weights into device layouts. p: dict of np arrays."""
    f32 = lambda x: np.ascontiguousarray(x, dtype=np.float32)
    C = np.eye(64, dtype=np.float64) - 1.0 / 64.0

    c = {}
    c["W1c"] = f32(p["W1"].astype(np.float64) @ C)  # [128, 64]
    c["b1c"] = f32(p["b1"].astype(np.float64) @ C)  # [64]
    c["W2c"] = f32(p["W2"].astype(np.float64) @ C)  # [96, 64]
    c["b2c"] = f32(p["b2"].astype(np.float64) @ C)  # [64]
    c["g1be1"] = f32(np.stack([np.tile(p["g1"], 2), np.tile(p["be1"], 2)], 1))
    c["g2be2"] = f32(np.stack([np.tile(p["g2"], 2), np.tile(p["be2"], 2)], 1))

    # Gate weights: lhsT rows = xh rows = [h (64); x (64)]; reference W rows =
    # [x (64); h (64)].  Columns i|j|f|o stay in place.
    c["Wgf"] = f32(np.concatenate([p["Wf"][64:128], p["Wf"][0:64]], 0))
    c["Wgb"] = f32(np.concatenate([p["Wb"][64:128], p["Wb"][0:64]], 0))
    gb = np.zeros((128, 4), np.float64)
    for gi, sl in enumerate((slice(0, 64), slice(64, 128), slice(128, 192), slice(192, 256))):
        gb[0:64, gi] = p["bf"][sl]
        gb[64:128, gi] = p["bb"][sl]
    gb[:, 2] += 1.0  # forget_bias
    c["gbias"] = f32(gb)  # cols: i, j, f, o

    c["whead"] = f32(np.stack([p["wx"], p["wp"]], 1))  # [64, 2]
    c["CT"] = f32(np.eye(64) - 1.0 / 64.0)
    c["ones_red"] = f32(
        np.block(
            [[np.ones((64, 1)), np.zeros((64, 1))], [np.zeros((64, 1)), np.ones((64, 1))]]
        )
    )  # [128, 2]
    c["ones_rep"] = f32(
        np.block(
            [[np.ones((1, 64)), np.zeros((1, 64))], [np.zeros((1, 64)), np.ones((1, 64))]]
        )
    )  # [2, 128]
    c["ones64"] = f32(np.ones((64, 1)))
    c["ones_1_64"] = f32(np.ones((1, 64)))

    # Head params; bw direction is time-reversed relative to our bw scan
    # state order, handled by reversing the per-t parameters.
    c["W3f"] = f32(p["W3"])
    c["W3b"] = f32(p["W3"][::-1, ::-1])
    c["b3f"] = f32(p["b3"].reshape(64, 1))
    c["b3b"] = f32(p["b3"][::-1].reshape(64, 1))
    c["gpbepf"] = f32(np.stack([p["gp"], p["bep"]], 1))  # [64, 2]
    c["gpbepb"] = f32(np.stack([p["gp"][::-1], p["bep"][::-1]], 1))
    c["bx"] = float(np.asarray(p["bx"]))
    return c


# ------------------------------------------------------------ bass program --


def _build(consts):
    nc = bass.Bass()
    obsT = nc.declare_dram_parameter("obsT", [128, NT], F32, isOutput=False)
    actT = nc.declare_dram_parameter("actT", [32, NT], F32, isOutput=False)

    cin = {}
    for name in (
        "W1c", "b1c", "W2c", "b2c", "g1be1", "g2be2", "Wgf", "Wgb", "gbias",
        "whead", "CT", "ones_red", "ones_rep", "ones64", "ones_1_64",
        "W3f", "W3b", "b3f", "b3b", "gpbepf", "gpbepb",
    ):
        a = consts[name]
        shp = list(a.shape) if a.ndim == 2 else [a.shape[0], 1]
        cin[name] = nc.declare_dram_parameter(name, shp, F32, isOutput=False)

    ov = nc.declare_dram_parameter("ov", [2, BC], F32, isOutput=True)
    xs_dram = nc.dram_tensor("xs_stash", [4, NT], F32)

    b1_nz = bool(np.any(consts["b1c"]))
    b2_nz = bool(np.any(consts["b2c"]))

    with tile.TileContext(nc) as tc:
        with (
            tc.tile_pool(name="singles", bufs=1) as sing,
            tc.tile_pool(name="obs_p", bufs=4) as obs_p,
            tc.tile_pool(name="act_p", bufs=4) as act_p,
            tc.tile_pool(name="big", bufs=2) as big,
            tc.tile_pool(name="small", bufs=4) as small,
        ):
            # ---- load constants ----
            ct = {}
            for name, dram in cin.items():
                a = consts[name]
                shp = list(a.shape) if a.ndim == 2 else [a.shape[0], 1]
                ct[name] = sing.tile(shp, F32, name=f"ct_{name}", tag=f"ct_{name}")
                nc.sync.dma_start(out=ct[name], in_=dram[:, :])

            X2 = sing.tile([128, 32 * BC], F32)  # x2, two-ended t-pair packing
            epst = sing.tile([128, 1], F32)
            nc.vector.memset(epst, EPS)

            # b1c/b2c as [128,1] per-partition (2-stacked) if needed
            if b1_nz or b2_nz:
                bstk = sing.tile([128, 2], F32)
                # col 0 = [b1c;b1c], col 1 = [b2c;b2c] via DMA broadcast: the
                # host passes b1c/b2c as [64,1]; replicate by two DMAs each.
                for col, nm in ((0, "b1c"), (1, "b2c")):
                    nc.sync.dma_start(out=bstk[0:64, col : col + 1], in_=cin[nm][:, :])
                    nc.sync.dma_start(out=bstk[64:128, col : col + 1], in_=cin[nm][:, :])

            # ================= Phase T: trunk =================
            trunk_ps = tc.tile_pool(name="trunk_ps", bufs=2, space="PSUM")
            with trunk_ps as ps_v, tc.tile_pool(name="trunk_ps2", bufs=2, space="PSUM") as ps_s, tc.tile_pool(name="trunk_ps3", bufs=2, space="PSUM") as ps_r:
              for j in range(32):
                  ta, tb = j, 63 - j
                  oa = obs_p.tile([128, BC], F32, tag="obs")
                  nc.sync.dma_start(out=oa, in_=obsT[:, ta * BC : (ta + 1) * BC])
                  ob = obs_p.tile([128, BC], F32, tag="obs")
                  nc.sync.dma_start(out=ob, in_=obsT[:, tb * BC : (tb + 1) * BC])

                  # ---- layer 1 ----
                  pv = ps_v.tile([128, BC], F32, tag="pv")
                  nc.tensor.matmul(pv[0:64, :], ct["W1c"], oa, start=True, stop=True,
                                   tile_position=(0, 0))
                  nc.tensor.matmul(pv[64:128, :], ct["W1c"], ob, start=True, stop=True,
                                   tile_position=(0, 64))
                  vsq = big.tile([128, BC], F32, tag="vsq")
                  v_s = big.tile([128, BC], F32, tag="v_s")
                  if b1_nz:
                      nc.scalar.activation(vsq, pv, AF.Square, bias=bstk[:, 0:1])
                      nc.scalar.activation(v_s, pv, AF.Identity, bias=bstk[:, 0:1])
                  else:
                      nc.scalar.activation(vsq, pv, AF.Square)
                      nc.vector.tensor_copy(v_s, pv)
                  pss = ps_s.tile([2, BC], F32, tag="pss")
                  nc.tensor.matmul(pss, ct["ones_red"], vsq, start=True, stop=True)
                  stdv = small.tile([2, BC], F32, tag="stdv")
                  nc.scalar.activation(stdv, pss, AF.Sqrt, bias=epst[0:2, 0:1], scale=1.0 / 64.0)
                  rstd = small.tile([2, BC], F32, tag="rstd")
                  nc.vector.reciprocal(rstd, stdv)
                  prep = ps_r.tile([128, BC], F32, tag="prep")
                  nc.tensor.matmul(prep, ct["ones_rep"], rstd, start=True, stop=True)
                  xn = big.tile([128, BC], F32, tag="xn")
                  nc.vector.tensor_mul(xn, v_s, prep)
                  xa = big.tile([96, BC], F32, tag="xa")
                  xb = big.tile([96, BC], F32, tag="xb")
                  nc.scalar.activation(xa[0:64, :], xn[0:64, :], AF.Relu,
                                       bias=ct["g1be1"][0:64, 1:2],
                                       scale=ct["g1be1"][0:64, 0:1])
                  nc.scalar.activation(xb[0:64, :], xn[64:128, :], AF.Relu,
                                       bias=ct["g1be1"][64:128, 1:2],
                                       scale=ct["g1be1"][64:128, 0:1])
                  nc.sync.dma_start(out=xa[64:96, :], in_=actT[:, ta * BC : (ta + 1) * BC])
                  nc.sync.dma_start(out=xb[64:96, :], in_=actT[:, tb * BC : (tb + 1) * BC])

                  # ---- layer 2 ----
                  pv2 = ps_v.tile([128, BC], F32, tag="pv")
                  nc.tensor.matmul(pv2[0:64, :], ct["W2c"], xa[0:96, :], start=True,
                                   stop=True, tile_position=(0, 0))
                  nc.tensor.matmul(pv2[64:128, :], ct["W2c"], xb[0:96, :], start=True,
                                   stop=True, tile_position=(0, 64))
                  vsq2 = big.tile([128, BC], F32, tag="vsq")
                  v_s2 = big.tile([128, BC], F32, tag="v_s")
                  if b2_nz:
                      nc.scalar.activation(vsq2, pv2, AF.Square, bias=bstk[:, 1:2])
                      nc.scalar.activation(v_s2, pv2, AF.Identity, bias=bstk[:, 1:2])
                  else:
                      nc.scalar.activation(vsq2, pv2, AF.Square)
                      nc.vector.tensor_copy(v_s2, pv2)
                  pss2 = ps_s.tile([2, BC], F32, tag="pss")
                  nc.tensor.matmul(pss2, ct["ones_red"], vsq2, start=True, stop=True)
                  stdv2 = small.tile([2, BC], F32, tag="stdv")
                  nc.scalar.activation(stdv2, pss2, AF.Sqrt, bias=epst[0:2, 0:1], scale=1.0 / 64.0)
                  rstd2 = small.tile([2, BC], F32, tag="rstd")
                  nc.vector.reciprocal(rstd2, stdv2)
                  prep2 = ps_r.tile([128, BC], F32, tag="prep")
                  nc.tensor.matmul(prep2, ct["ones_rep"], rstd2, start=True, stop=True)
                  xn2 = big.tile([128, BC], F32, tag="xn")
                  nc.vector.tensor_mul(xn2, v_s2, prep2)
                  # write both t-blocks of x2 into X2 block j in one op
                  nc.scalar.activation(X2[:, j * BC : (j + 1) * BC], xn2, AF.Relu,
                                       bias=ct["g2be2"][:, 1:2],
                                       scale=ct["g2be2"][:, 0:1])

            tc.strict_bb_all_engine_barrier()

            # ================= Phase L: LSTM =================
            xh_f = sing.tile([128, BC], F32)  # rows 0:64 h_fw, 64:128 x_fw
            xh_b = sing.tile([128, BC], F32)
            cst = sing.tile([128, BC], F32)  # cell state [c_fw; c_bw]
            nc.vector.memset(xh_f[0:64, :], 0.0)
            nc.vector.memset(xh_b[0:64, :], 0.0)
            nc.vector.memset(cst, 0.0)

            lstm_ps_cm = tc.tile_pool(name="lstm_ps", bufs=1, space="PSUM")
            lstm_ps = lstm_ps_cm.__enter__()
            pg_i = lstm_ps.tile([128, BC], F32, tag="pg_i")
            pg_j = lstm_ps.tile([128, BC], F32, tag="pg_j")
            pg_f = lstm_ps.tile([128, BC], F32, tag="pg_f")
            pg_o = lstm_ps.tile([128, BC], F32, tag="pg_o")
            ph_f = [lstm_ps.tile([2, 2 * BC], F32, name=f"ph_f{i}", tag=f"ph_f{i}") for i in range(1)]
            ph_b = [lstm_ps.tile([2, 2 * BC], F32, name=f"ph_b{i}", tag=f"ph_b{i}") for i in range(1)]
            stg_f = [sing.tile([2, 2 * BC], F32, name=f"stg_f{i}", tag=f"stg_f{i}") for i in range(2)]
            stg_b = [sing.tile([2, 2 * BC], F32, name=f"stg_b{i}", tag=f"stg_b{i}") for i in range(2)]

            def xsrc(t_needed):
                if t_needed < 32:
                    return X2[0:64, t_needed * BC : (t_needed + 1) * BC]
                jj = 63 - t_needed
                return X2[64:128, jj * BC : (jj + 1) * BC]

            gates = ((pg_i, 0, AF.Sigmoid), (pg_j, 1, AF.Tanh),
                     (pg_f, 2, AF.Sigmoid), (pg_o, 3, AF.Sigmoid))

            for t in range(T):
                nc.vector.tensor_copy(xh_f[64:128, :], xsrc(t))
                nc.vector.tensor_copy(xh_b[64:128, :], xsrc(63 - t))
                for pg, gi, _fn in gates:
                    wf = ct["Wgf"][:, gi * 64 : (gi + 1) * 64]
                    wb = ct["Wgb"][:, gi * 64 : (gi + 1) * 64]
                    nc.tensor.matmul(pg[0:64, :], wf, xh_f, start=True, stop=True,
                                     tile_position=(0, 0))
                    nc.tensor.matmul(pg[64:128, :], wb, xh_b, start=True, stop=True,
                                     tile_position=(0, 64))
                sI = big.tile([128, BC], F32, tag="sI")
                tJ = big.tile([128, BC], F32, tag="tJ")
                sF = big.tile([128, BC], F32, tag="sF")
                sO = big.tile([128, BC], F32, tag="sO")
                for (pg, gi, fn), dst in zip(gates, (sI, tJ, sF, sO)):
                    nc.scalar.activation(dst, pg, fn, bias=ct["gbias"][:, gi : gi + 1])
                u = big.tile([128, BC], F32, tag="u")
                nc.vector.tensor_mul(u, sI, tJ)
                cf = big.tile([128, BC], F32, tag="cf")
                nc.vector.tensor_mul(cf, cst, sF)
                nc.vector.tensor_add(cst, cf, u)
                tcl = big.tile([128, BC], F32, tag="tc")
                nc.scalar.activation(tcl, cst, AF.Tanh)
                nc.vector.tensor_mul(xh_f[0:64, :], tcl[0:64, :], sO[0:64, :])
                nc.vector.tensor_mul(xh_b[0:64, :], tcl[64:128, :], sO[64:128, :])

                # head dots: [xs; ps] rows for this step
                ph = ph_f[0]
                pb = ph_b[0]
                col = (t % 2) * BC
                nc.tensor.matmul(ph[0:2, col : col + BC], ct["whead"], xh_f[0:64, :],
                                 start=True, stop=True, tile_position=(0, 0))
                nc.tensor.matmul(pb[0:2, col : col + BC], ct["whead"], xh_b[0:64, :],
                                 start=True, stop=True, tile_position=(0, 0))
                if t % 2 == 1:
                    sf = stg_f[(t // 2) % 2]
                    sb = stg_b[(t // 2) % 2]
                    nc.vector.tensor_copy(sf, ph)
                    nc.vector.tensor_copy(sb, pb)
                    dcol = (t - 1) * BC
                    nc.sync.dma_start(out=xs_dram[0:2, dcol : dcol + 2 * BC], in_=sf)
                    nc.sync.dma_start(out=xs_dram[2:4, dcol : dcol + 2 * BC], in_=sb)

            lstm_ps_cm.__exit__(None, None, None)
            tc.strict_bb_all_engine_barrier()

            # ================= Phase H: head =================
            head_ps_cm = tc.tile_pool(name="head_ps", bufs=1, space="PSUM")
            ps_h = head_ps_cm.__enter__()
            pn_d = []
            xs_d = []
            for d, (w3, b3, gpb) in enumerate(
                (("W3f", "b3f", "gpbepf"), ("W3b", "b3b", "gpbepb"))
            ):
                xsT = big.tile([64, BC], F32, tag="u")
                psT = big.tile([64, BC], F32, tag="cf")
                nc.sync.dma_start(
                    out=xsT, in_=xs_dram[2 * d : 2 * d + 1, :].rearrange("o (t b) -> (o t) b", b=BC)
                )
                nc.sync.dma_start(
                    out=psT, in_=xs_dram[2 * d + 1 : 2 * d + 2, :].rearrange("o (t b) -> (o t) b", b=BC)
                )
                pc = ps_h.tile([64, BC], F32, tag="hpc")
                nc.tensor.matmul(pc, ct["CT"], psT, start=True, stop=True)
                hsq = big.tile([64, BC], F32, tag="vsq")
                hcs = big.tile([64, BC], F32, tag="v_s")
                nc.scalar.activation(hsq, pc, AF.Square)
                nc.vector.tensor_copy(hcs, pc)
                hss = ps_h.tile([1, BC], F32, tag="hss")
                nc.tensor.matmul(hss, ct["ones64"], hsq, start=True, stop=True)
                hstd = small.tile([1, BC], F32, tag="stdv")
                nc.scalar.activation(hstd, hss, AF.Sqrt, bias=epst[0:1, 0:1], scale=1.0 / 64.0)
                hrst = small.tile([1, BC], F32, tag="rstd")
                nc.vector.reciprocal(hrst, hstd)
                hrep = ps_h.tile([64, BC], F32, tag="hrep")
                nc.tensor.matmul(hrep, ct["ones_1_64"], hrst, start=True, stop=True)
                ht1 = big.tile([64, BC], F32, tag="xn")
                nc.vector.tensor_mul(ht1, hcs, hrep)
                pn = big.tile([64, BC], F32, tag="tc")
                nc.scalar.activation(pn, ht1, AF.Relu, bias=ct[gpb][:, 1:2],
                                     scale=ct[gpb][:, 0:1])
                pn_d.append((pn, w3, b3))
                xs_d.append(xsT)

            ovs0 = sing.tile([1, BC], F32)
            ovs1 = sing.tile([1, BC], F32)
            for d, ((pn, w3, b3), xsT) in enumerate(zip(pn_d, xs_d)):
                pl = ps_h.tile([64, BC], F32, tag="hpl")
                nc.tensor.matmul(pl, ct[w3], pn, start=True, stop=True)
                he = big.tile([64, BC], F32, tag="sI")
                nc.scalar.activation(he, pl, AF.Exp, bias=ct[b3][:, 0:1])
                hse = ps_h.tile([1, BC], F32, tag="hse")
                nc.tensor.matmul(hse, ct["ones64"], he, start=True, stop=True)
                hrs = small.tile([1, BC], F32, tag="rstd")
                nc.vector.reciprocal(hrs, hse)
                hex = big.tile([64, BC], F32, tag="tJ")
                nc.vector.tensor_mul(hex, he, xsT)
                hnum = ps_h.tile([1, BC], F32, tag="hnum")
                nc.tensor.matmul(hnum, ct["ones64"], hex, start=True, stop=True)
                hov = small.tile([1, BC], F32, tag="stdv")
                nc.vector.tensor_mul(hov, hrs, hnum)
                nc.vector.tensor_scalar(
                    ovs0 if d == 0 else ovs1, hov, float(consts["bx"]), None, ALU.add
                )
            nc.sync.dma_start(out=ov[0:1, :], in_=ovs0)
            nc.sync.dma_start(out=ov[1:2, :], in_=ovs1)
            head_ps_cm.__exit__(None, None, None)

    return nc


_CACHE = {}

# test.py hooks: set TRACE=True before calling kernel() to capture an NTFF
# profile; the full BassKernelResults lands in LAST_RESULT.
TRACE = False
LAST_RESULT = None


def kernel(**inputs):
    obs = np.ascontiguousarray(inputs["obs"], dtype=np.float32)
    action = np.ascontiguousarray(inputs["action"], dtype=np.float32)
    consts = _prep_consts(inputs)

    key = "nc"
    if key not in _CACHE:
        _CACHE[key] = _install_waitfix(_build(consts))
    nc = _CACHE[key]

    const_feed = {}
    for name, a in consts.items():
        if name == "bx":
            continue
        const_feed[name] = a if a.ndim == 2 else a.reshape(-1, 1)

    in_maps = []
    for c in range(NCORES):
        sl = slice(c * BC * T, (c + 1) * BC * T)
        obsT = np.ascontiguousarray(
            obs[sl].reshape(BC, T, OBS).transpose(2, 1, 0).reshape(OBS, NT)
        )
        actT = np.ascontiguousarray(
            action[sl].reshape(BC, T, ACTD).transpose(2, 1, 0).reshape(ACTD, NT)
        )
        m = {"obsT": obsT, "actT": actT}
        m.update(const_feed)
        in_maps.append(m)

    global LAST_RESULT
    kw = {}
    if TRACE:
        import tempfile

        kw = dict(trace=True, tmpdir=tempfile.mkdtemp(prefix="bass_trace_"))
    LAST_RESULT = run_bass_kernel_spmd(nc, in_maps, list(range(NCORES)), **kw)
    res = LAST_RESULT.results

    out = np.empty(2 * B, dtype=np.float32)
    for c in range(NCORES):
        ovc = res[c]["ov"]
        out[c * BC : (c + 1) * BC] = ovc[0]
        out[B + c * BC : B + (c + 1) * BC] = ovc[1]
    return out
